# revision 23
# baseline (speedup 1.0000x reference)
"""Trainium2 kernel for nn_GroupoidDecompositionLayer.

Reference computes out = (tensor @ W @ basis)[:, 0], which factors as
    out = tensor @ (W @ basis[:, 0])
i.e. two chained matvecs.  Work is DMA-bound (tensor 128MB + W 48MB reads),
so we shard the contraction dim (4096) across the 8 cores:

  core i gets tensor[:, 512i:512(i+1)] and W[512i:512(i+1), :],
  computes v_i = W_i @ b0 then p_i = T_i @ v_i on the TensorEngine,
  host sums the 8 partial outputs (the gather step).

Operands are stored fp16 (halves DMA, the bottleneck); all accumulation is
f32 in PSUM, so products of fp16 values are exact and the end-to-end error
stays ~4e-4 relative.  All device operands are laid out on the host so every
DMA is contiguous per partition.
"""

import numpy as np

import concourse.tile as tile
from concourse import bacc, mybir
from concourse.bass_utils import run_bass_kernel_spmd

BATCH = 8192   # tensor rows
KDIM = 4096    # contraction dim (tensor cols == W rows)
JDIM = 3072    # W cols == basis rows
NCORES = 8
KS = KDIM // NCORES          # 512 contraction cols per core
KT = KS // 128               # 4 k-tiles of 128 partitions (phase 2)
JT = JDIM // 128             # 24 j-tiles of 128 partitions (phase 1)
MT = BATCH // 128            # 64 output chunks of 128
CH = 4096                    # tensor free-dim DMA chunk (1MB fp16 tiles)
NG = BATCH // CH             # 2 chunk groups
TPG = CH // 128              # 32 output chunks per group
NWC = 3                      # wt DMA chunks (1MB each)

F32 = mybir.dt.float32
F16 = mybir.dt.float16
NP_STORE = np.float16


def _build_nc(fine_tail=True, psum_split=True, out_split=True, tt_first=True):
    nc = bacc.Bacc("TRN2", target_bir_lowering=False, debug=False,
                   num_devices=NCORES)

    # tt:  tensor slice, pre-transposed on host -> [KS, BATCH]
    # wtp: W slice, packed so partition r, col kk*KS+c == W_i[c, 128kk+r]
    # b0p: basis[:,0], packed so partition r, col kk == b0[128kk+r]
    tt = nc.dram_tensor("tt", [KS, BATCH], F16, kind="ExternalInput")
    wtp = nc.dram_tensor("wtp", [128, JT * KS], F16, kind="ExternalInput")
    b0p = nc.dram_tensor("b0p", [128, JT], F16, kind="ExternalInput")
    # out[r, t] == p[128t + r]
    out = nc.dram_tensor("out", [128, MT], F32, kind="ExternalOutput")

    with tile.TileContext(nc) as tc:
        with (
            tc.tile_pool(name="const", bufs=1) as const,
            tc.tile_pool(name="ttp", bufs=NG * KT) as ttp,
            tc.tile_pool(name="psum", bufs=1, space="PSUM") as psum,
        ):
            # DMA issue order drives HWDGE descriptor-gen order: a tensor
            # tile goes absolutely first so HBM bytes start moving ASAP;
            # the W-path (phase 1) fits easily in the slack behind it.
            # The final k-row is chunked fine so only a handful of matmuls
            # trail the last transfer.
            chunks = {kk: [(0, CH), (CH, CH)] for kk in range(KT)}
            if fine_tail:
                chunks[KT - 1] = [(c0, 1024) for c0 in range(0, BATCH, 1024)]
            tt_tiles = {}

            def dma_tt(kk, c0, w, eng=None):
                t_ = ttp.tile([128, w], F16, tag=f"tt{w}")
                (eng or nc.sync).dma_start(
                    t_[:], tt[128 * kk:128 * (kk + 1), c0:c0 + w])
                tt_tiles[(kk, c0)] = (t_, c0, w)

            if tt_first:
                dma_tt(0, 0, CH)

            b0_t = const.tile([128, JT], F16, tag="b0")
            nc.sync.dma_start(b0_t[:], b0p[:])

            wt_t = const.tile([128, JT * KS], F16, tag="wt")
            wchunk = JT * KS // NWC
            for g in range(NWC):
                nc.sync.dma_start(wt_t[:, g * wchunk:(g + 1) * wchunk],
                                  wtp[:, g * wchunk:(g + 1) * wchunk])

            # remaining tensor tiles, kk-major so late-kk tiles arrive last
            for kk in range(KT):
                for c0, w in chunks[kk]:
                    if (kk, c0) not in tt_tiles:
                        dma_tt(kk, c0, w)

            # ---- phase 1: v = W_i @ b0 ----------------------------------
            # vpsum[c', mv] = v[128mv + c']
            vpsum = psum.tile([128, KT], F32, tag="vps")
            for mv in range(KT):
                for kk in range(JT):
                    lo = kk * KS + 128 * mv
                    nc.tensor.matmul(
                        vpsum[:, mv:mv + 1],
                        wt_t[:, lo:lo + 128],
                        b0_t[:, kk:kk + 1],
                        start=(kk == 0), stop=(kk == JT - 1),
                    )
            v_sb = const.tile([128, KT], F16, tag="vsb")
            nc.vector.tensor_copy(v_sb[:], vpsum[:])

            # ---- phase 2: p = T_i @ v ----------------------------------
            # kk-outer: column t's accumulation group stays open across the
            # kk sweep, so after the final tile lands only its own 32
            # matmuls remain (instead of a whole group's 128).
            # two PSUM banks: columns 0..55 and 56..63 — lets the DVE
            # evacuate bank A while the PE still writes bank B (same-bank
            # PE-write/DVE-read pairs are serialized by Tile)
            TSPLIT = MT - 8 if psum_split else MT
            ppsum_a = psum.tile([128, TSPLIT], F32, tag="ppsA")
            if psum_split:
                ppsum_b = psum.tile([128, MT - TSPLIT], F32, tag="ppsB")
            else:
                ppsum_b = None
            # t-outer: each column's 4-matmul accumulation group is
            # contiguous (interleaved groups in one PSUM zero region are
            # illegal); the PE just stalls inside a group until that
            # column's late chunk lands, which is free at 2ns dispatch
            def chunk_of(kk, t):
                for c0, w in chunks[kk]:
                    if c0 <= 128 * t < c0 + w:
                        return tt_tiles[(kk, c0)][0], 128 * t - c0
                raise AssertionError

            for t in range(MT):
                dst = (ppsum_a[:, t:t + 1] if t < TSPLIT
                       else ppsum_b[:, t - TSPLIT:t - TSPLIT + 1])
                for kk in range(KT):
                    t_, col = chunk_of(kk, t)
                    nc.tensor.matmul(
                        dst,
                        t_[:, col:col + 128],
                        v_sb[:, kk:kk + 1],
                        start=(kk == 0), stop=(kk == KT - 1),
                    )

            # evacuate the first 56 columns early so their DMA start +
            # HBM-completion latency hides under the final chunk's matmuls;
            # only the last 8 columns' tiny copy+DMA trails the last matmul.
            # Out DMAs ride the ACT HWDGE ring (no queueing behind inputs).
            out_sb = const.tile([128, MT], F32, tag="osb")
            if out_split and psum_split:
                nc.vector.tensor_copy(out_sb[:, 0:TSPLIT], ppsum_a[:])
                nc.sync.dma_start(out[:, 0:TSPLIT], out_sb[:, 0:TSPLIT])
                nc.vector.tensor_copy(out_sb[:, TSPLIT:MT], ppsum_b[:])
                nc.sync.dma_start(out[:, TSPLIT:MT], out_sb[:, TSPLIT:MT])
            else:
                nc.vector.tensor_copy(out_sb[:, 0:TSPLIT], ppsum_a[:])
                if psum_split:
                    nc.vector.tensor_copy(out_sb[:, TSPLIT:MT], ppsum_b[:])
                nc.sync.dma_start(out[:], out_sb[:])

    nc.compile()
    return nc


def _shard_inputs(tensor, W, basis):
    b0 = np.ascontiguousarray(
        basis[:, 0].reshape(JT, 128).T).astype(NP_STORE)   # [128, JT]
    # tt_all[i, c, m] = tensor[m, KS*i + c]
    tt_all = np.ascontiguousarray(
        tensor.astype(NP_STORE).reshape(BATCH, NCORES, KS).transpose(1, 2, 0))
    # wt_all[i, r, kk, c] = W[KS*i + c, 128kk + r]
    wt_all = np.ascontiguousarray(
        W.astype(NP_STORE).reshape(NCORES, KS, JT, 128).transpose(0, 3, 2, 1)
    ).reshape(NCORES, 128, JT * KS)
    return [{"tt": tt_all[i], "wtp": wt_all[i], "b0p": b0}
            for i in range(NCORES)]


_NC_CACHE = []


def kernel(tensor: np.ndarray, W: np.ndarray, basis: np.ndarray) -> np.ndarray:
    tensor = np.asarray(tensor, dtype=np.float32)
    W = np.asarray(W, dtype=np.float32)
    basis = np.asarray(basis, dtype=np.float32)

    if not _NC_CACHE:
        _NC_CACHE.append(_build_nc())
    nc = _NC_CACHE[0]
    in_maps = _shard_inputs(tensor, W, basis)
    res = run_bass_kernel_spmd(nc, in_maps, core_ids=list(range(NCORES)))

    out = np.zeros(BATCH, dtype=np.float32)
    for i in range(NCORES):
        out += res.results[i]["out"].T.reshape(BATCH)
    return out
